# revision 1
# baseline (speedup 1.0000x reference)
"""Trainium2 Bass kernel for a ViT-style transformer block (B=64,N=197,C=768,H=12,P=20).

Strategy: data-parallel over batch across 8 NeuronCores (8 images/core).
Inside each core: feature-major activations, f32r matmuls for the big GEMMs,
bf16 attention, LN via ones-matmul column sums + gpsimd partition broadcast,
attention with kv-on-partitions scores (S_T = K^T q) so no on-chip transposes
are needed, softmax denominator from an appended ones-column on token-major V.
"""
import numpy as np
import concourse.bass as bass
import concourse.mybir as mybir
import concourse.tile as tile
from concourse import bacc, bass_utils
from contextlib import ExitStack

F32 = mybir.dt.float32
F32R = mybir.dt.float32r
BF16 = mybir.dt.bfloat16
AF = mybir.ActivationFunctionType

B, N, C, H, Dh, P, Dff = 64, 197, 768, 12, 64, 20, 3072
NCORES = 8
BL = B // NCORES          # images per core
KV = P + N                # 217
CT = C // 128             # 6 c-tiles
QP = N + 1                # 198, even padded q width
EPS = 1e-6


def build_nc(bl=BL):
    T = bl * N
    TP = T + 8                 # padded q columns
    nch = max(bl // 2, 1)      # chunks of 2 images
    chunk = T // nch           # 394 (even, >=256)

    nc = bacc.Bacc(trn_type="TRN2", target_bir_lowering=False)

    x_fm = nc.dram_tensor("x_fm", [C, T], F32R, kind="ExternalInput")
    kp = nc.dram_tensor("kp", [CT, 128, bl, P], BF16, kind="ExternalInput")
    vp = nc.dram_tensor("vp", [bl, P, H, Dh], BF16, kind="ExternalInput")
    w_qk = nc.dram_tensor("w_qk", [C, 2 * C], F32R, kind="ExternalInput")
    b_qk = nc.dram_tensor("b_qk", [128, 12], F32, kind="ExternalInput")
    w_v = nc.dram_tensor("w_v", [C, C], F32R, kind="ExternalInput")
    b_v = nc.dram_tensor("b_v", [128, 6], F32, kind="ExternalInput")
    w_pr = nc.dram_tensor("w_pr", [C, C], BF16, kind="ExternalInput")
    b_pr = nc.dram_tensor("b_pr", [128, 6], F32, kind="ExternalInput")
    w_f1 = nc.dram_tensor("w_f1", [C, Dff], F32R, kind="ExternalInput")
    b_f1 = nc.dram_tensor("b_f1", [128, 24], F32, kind="ExternalInput")
    w_f2 = nc.dram_tensor("w_f2", [Dff, C], BF16, kind="ExternalInput")
    b_f2 = nc.dram_tensor("b_f2", [128, 6], F32, kind="ExternalInput")
    out_fm = nc.dram_tensor("out_fm", [C, T], F32, kind="ExternalOutput")

    with tile.TileContext(nc) as tc, ExitStack() as top:
        consts = top.enter_context(tc.tile_pool(name="consts", bufs=1))
        ones_f = consts.tile([128, 8], F32)
        nc.vector.memset(ones_f[:], 1.0)
        zeros_f = consts.tile([128, 8], F32)
        nc.vector.memset(zeros_f[:], 0.0)
        ones_r = consts.tile([128, 1], F32R)
        nc.vector.tensor_copy(ones_r[:], ones_f[:, 0:1])
        eps_sb = consts.tile([1, 1], F32)
        nc.vector.memset(eps_sb[:], EPS)
        bqk_sb = consts.tile([128, 12], F32)
        nc.sync.dma_start(bqk_sb[:], b_qk[:])
        bv_sb = consts.tile([128, 6], F32)
        nc.sync.dma_start(bv_sb[:], b_v[:])
        bpr_sb = consts.tile([128, 6], F32)
        nc.sync.dma_start(bpr_sb[:], b_pr[:])
        bf1_sb = consts.tile([128, 24], F32)
        nc.sync.dma_start(bf1_sb[:], b_f1[:])
        bf2_sb = consts.tile([128, 6], F32)
        nc.sync.dma_start(bf2_sb[:], b_f2[:])

        # single always-open PSUM pool (1 tag, 8 banks) and weights pool
        psA = top.enter_context(tc.tile_pool(name="psA", bufs=2, space="PSUM"))
        psB = top.enter_context(tc.tile_pool(name="psB", bufs=2, space="PSUM"))

        def psum(shape, name):
            return psA.tile(shape, F32, tag="mm", name=name)

        wpool = top.enter_context(tc.tile_pool(name="wpool", bufs=2))

        main = top.enter_context(tc.tile_pool(name="main", bufs=1))
        o_fm = [main.tile([128, T], BF16, tag=f"o{i}", name=f"o{i}") for i in range(CT)]
        x2 = [main.tile([128, T], F32R, tag=f"x2_{i}", name=f"x2_{i}") for i in range(CT)]

        def ln_stats(xs, lp, jtag):
            """Column mean/rstd for one chunk -> broadcast (a_bc, b_bc); xh = x*a_bc - b_bc."""
            sq = []
            for i in range(CT):
                sqt = lp.tile([128, chunk], F32R, tag=f"sq{i & 1}", name=f"sq{i & 1}")
                nc.vector.tensor_mul(sqt[:], xs[i][:], xs[i][:])
                sq.append(sqt)
            s1 = psum([1, chunk], "s1")
            s2 = psum([1, chunk], "s2")
            for i in range(CT):
                nc.tensor.matmul(s1[:], ones_r[:], xs[i][:], start=(i == 0), stop=(i == CT - 1))
            for i in range(CT):
                nc.tensor.matmul(s2[:], ones_r[:], sq[i][:], start=(i == 0), stop=(i == CT - 1))
            mu = lp.tile([1, chunk], F32R, tag="mu", name="mu")
            nc.vector.tensor_scalar_mul(mu[:], s1[:], 1.0 / C)
            musq = lp.tile([1, chunk], F32R, tag="musq", name="musq")
            nc.vector.tensor_mul(musq[:], mu[:], mu[:])
            var = lp.tile([1, chunk], F32R, tag="var", name="var")
            nc.vector.tensor_scalar_mul(var[:], s2[:], 1.0 / C)
            nc.vector.tensor_sub(var[:], var[:], musq[:])
            sd = lp.tile([1, chunk], F32R, tag="sd", name="sd")
            nc.scalar.activation(out=sd[:], in_=var[:], func=AF.Sqrt, bias=eps_sb[:])
            rs = lp.tile([1, chunk], F32R, tag="rs", name="rs")
            with nc.allow_low_precision(reason="f32r is fp32-width"):
                nc.vector.reciprocal(rs[:], sd[:])
            murs = lp.tile([1, chunk], F32R, tag="murs", name="murs")
            nc.vector.tensor_mul(murs[:], mu[:], rs[:])
            a_bc = lp.tile([128, chunk], F32R, tag=f"a_bc{jtag}", name=f"a_bc{jtag}")
            nc.gpsimd.partition_broadcast(a_bc[:], rs[:])
            b_bc = lp.tile([128, chunk], F32R, tag=f"b_bc{jtag}", name=f"b_bc{jtag}")
            nc.gpsimd.partition_broadcast(b_bc[:], murs[:])
            return a_bc, b_bc

        def ln_apply(xs, a_bc, b_bc, dst, j):
            for i in range(CT):
                d = dst[i][:, j * chunk:(j + 1) * chunk]
                nc.vector.tensor_mul(d, xs[i][:], a_bc[:])
                nc.vector.tensor_sub(d, d, b_bc[:])

        def layernorm_into(src_tiles_of, dst, label, bufs=2):
            with tc.tile_pool(name=f"ln_{label}", bufs=bufs) as lp:
                for j in range(nch):
                    xs = src_tiles_of(j)
                    a_bc, b_bc = ln_stats(xs, lp, "")
                    ln_apply(xs, a_bc, b_bc, dst, j)

        lnstash = top.enter_context(tc.tile_pool(name="lnstash", bufs=1))

        ph1 = ExitStack()
        xhpool = ph1.enter_context(tc.tile_pool(name="xhpool", bufs=1))
        xh = [xhpool.tile([128, TP], F32R, tag=f"xh{i}", name=f"xh{i}") for i in range(CT)]

        # ---------------- LN1 -> xh ----------------
        with tc.tile_pool(name="xload", bufs=2) as xlp:
            def load_x_chunk(j):
                ts = []
                for i in range(CT):
                    t = xlp.tile([128, chunk], F32R, tag=f"x{i}", name=f"x{i}")
                    nc.sync.dma_start(t[:], x_fm[i * 128:(i + 1) * 128, j * chunk:(j + 1) * chunk])
                    ts.append(t)
                return ts
            layernorm_into(load_x_chunk, xh, "ln1", bufs=2)

        for i in range(CT):
            nc.vector.tensor_copy(xh[i][:, T:TP], zeros_f[:, 0:TP - T])

        # ---------------- q/k GEMM + V GEMM ----------------
        persist = ph1.enter_context(tc.tile_pool(name="qkvpool", bufs=1))
        q_sb = [persist.tile([128, TP], BF16, tag=f"q{i}", name=f"q{i}") for i in range(CT)]
        k_sb = [persist.tile([128, bl, KV], BF16, tag=f"k{i}", name=f"k{i}") for i in range(CT)]
        v_sb = [[persist.tile([128, H, Dh + 1], BF16, tag=f"v{im}_{pt}", name=f"v{im}_{pt}")
                 for pt in range(2)] for im in range(bl)]

        for mt in range(12):
            wt = wpool.tile([128, CT, 128], F32R, tag="w", name="w")
            nc.sync.dma_start(
                wt[:], w_qk.rearrange("(kt p) m -> p kt m", p=128)[:, :, mt * 128:(mt + 1) * 128])
            for j in range(nch):
                ps = psum([128, chunk], "ps")
                for i in range(CT):
                    nc.tensor.matmul(ps[:], wt[:, i, :], xh[i][:, j * chunk:(j + 1) * chunk],
                                     start=(i == 0), stop=(i == CT - 1))
                if mt < 6:
                    nc.vector.tensor_scalar_add(
                        q_sb[mt][:, j * chunk:(j + 1) * chunk], ps[:], bqk_sb[:, mt:mt + 1])
                else:
                    for v in range(chunk // N):
                        im = j * (chunk // N) + v
                        nc.vector.tensor_scalar_add(
                            k_sb[mt - 6][:, im, 0:N], ps[:, v * N:(v + 1) * N],
                            bqk_sb[:, mt:mt + 1])
        with tc.tile_pool(name="wvp", bufs=1) as wvp:
            for nt2 in range(2):
                wv_t = []
                for i in range(CT):
                    wti = wvp.tile([128, 384], F32R, tag=f"wv{i}", name=f"wv{i}")
                    nc.sync.dma_start(wti[:], w_v[i * 128:(i + 1) * 128, nt2 * 384:(nt2 + 1) * 384])
                    wv_t.append(wti)
                for im in range(bl):
                    for pt, (toff, tsz) in enumerate([(0, 128), (128, N - 128)]):
                        ps = psum([128, 384], "psv")
                        for i in range(CT):
                            nc.tensor.matmul(
                                ps[:tsz, :], xh[i][:, im * N + toff: im * N + toff + tsz],
                                wv_t[i][:], start=(i == 0), stop=(i == CT - 1))
                        nc.vector.tensor_copy(
                            v_sb[im][pt][:tsz, nt2 * 6:(nt2 + 1) * 6, 0:Dh],
                            ps[:tsz, :].rearrange("t (h d) -> t h d", d=Dh))
        for im in range(bl):
            nc.sync.dma_start(v_sb[im][1][N - 128:N - 128 + P, :, 0:Dh], vp[im])
            for pt in range(2):
                nc.vector.tensor_copy(
                    v_sb[im][pt][:, :, Dh:Dh + 1],
                    ones_f[:, 0:1].to_broadcast([128, H, 1]))
        for i in range(CT):
            nc.vector.tensor_copy(q_sb[i][:, T:TP], zeros_f[:, 0:TP - T])
        with tc.tile_pool(name="kstage", bufs=2) as ksp:
            for i in range(CT):
                kst = ksp.tile([128, bl, P], BF16, tag="kst", name="kst")
                nc.sync.dma_start(kst[:], kp[i])
                nc.vector.tensor_copy(k_sb[i][:, :, N:KV], kst[:])

        # ---------------- attention + proj interleaved per image pair ----------------
        wprp = ph1.enter_context(tc.tile_pool(name="wprp", bufs=1))
        wpr_t = []
        for mt in range(CT):
            wt = wprp.tile([128, CT, 128], BF16, tag=f"wpr{mt}", name=f"wpr{mt}")
            nc.sync.dma_start(
                wt[:], w_pr.rearrange("(kt p) m -> p kt m", p=128)[:, :, mt * 128:(mt + 1) * 128])
            wpr_t.append(wt)
        attn_ctx = ph1.enter_context(tc.tile_pool(name="attn", bufs=2))
        ln2ab = {}
        if True:
            ap = attn_ctx
            for im in range(bl):
                for h in range(H):
                    hp, hoff = h // 2, (h % 2) * 64
                    q_ap = q_sb[hp][hoff:hoff + 64, im * N: im * N + QP]
                    k_ap = k_sb[hp][hoff:hoff + 64, im, :]
                    s0 = psB.tile([128, QP], F32, tag="s0", name="s0")
                    nc.tensor.matmul(s0[:], k_ap[:, 0:128], q_ap, start=True, stop=True)
                    s1 = psB.tile([128, QP], F32, tag="s1", name="s1")
                    nc.tensor.matmul(s1[0:KV - 128, :], k_ap[:, 128:KV], q_ap,
                                     start=True, stop=True)
                    e = ap.tile([128, 2, QP], BF16, tag="e", name="e")
                    nc.scalar.activation(out=e[:, 0, :], in_=s0[:], func=AF.Exp,
                                         scale=Dh ** -0.5)
                    nc.scalar.activation(out=e[0:KV - 128, 1, :], in_=s1[0:KV - 128, :],
                                         func=AF.Exp, scale=Dh ** -0.5)
                    av = psB.tile([128, QP], F32, tag="av", name="av")
                    nc.tensor.matmul(av[0:Dh + 1, :], v_sb[im][0][:, h, :], e[:, 0, :],
                                     start=True, stop=False)
                    nc.tensor.matmul(av[0:Dh + 1, :], v_sb[im][1][0:KV - 128, h, :],
                                     e[0:KV - 128, 1, :], start=False, stop=True)
                    rr = ap.tile([1, QP], F32R, tag="rr", name="rr")
                    with nc.allow_low_precision(reason="f32r is fp32-width"):
                        nc.vector.reciprocal(rr[:], av[Dh:Dh + 1, :])
                    rb = ap.tile([64, QP], F32R, tag="rb", name="rb")
                    nc.gpsimd.partition_broadcast(rb[:], rr[:])
                    o_ap = o_fm[hp][hoff:hoff + 64, im * N:(im + 1) * N]
                    nc.vector.tensor_mul(o_ap, av[0:Dh, 0:N], rb[:, 0:N])
                    nc.vector.tensor_scalar_add(o_ap, o_ap, bv_sb[hoff:hoff + 64, hp:hp + 1])
                if im % 2 == 1:
                    j = im // 2
                    for mt in range(CT):
                        ps = psum([128, chunk], "psp")
                        for i in range(CT):
                            nc.tensor.matmul(ps[:], wpr_t[mt][:, i, :],
                                             o_fm[i][:, j * chunk:(j + 1) * chunk],
                                             start=(i == 0), stop=(i == CT - 1))
                        xr = wpool.tile([128, chunk], F32, tag="w", name="xr")
                        nc.sync.dma_start(
                            xr[:], x_fm[mt * 128:(mt + 1) * 128, j * chunk:(j + 1) * chunk].bitcast(F32))
                        d = x2[mt][:, j * chunk:(j + 1) * chunk]
                        nc.vector.tensor_add(d, ps[:], xr[:])
                        nc.vector.tensor_scalar_add(d, d, bpr_sb[:, mt:mt + 1])
                    ln2ab[j] = ln_stats([x2[i][:, j * chunk:(j + 1) * chunk] for i in range(CT)],
                                        lnstash, f"_{j}")

        ph1.close()

        # ---------------- LN2 -> xh2, then MLP + residual -> out ----------------
        with ExitStack() as mlp:
            mpool = mlp.enter_context(tc.tile_pool(name="mlp", bufs=1))
            xh2 = [mpool.tile([128, T], F32R, tag=f"xh2_{i}", name=f"xh2_{i}") for i in range(CT)]
            for j in range(nch):
                a_bc, b_bc = ln2ab[j]
                ln_apply([x2[i][:, j * chunk:(j + 1) * chunk] for i in range(CT)],
                         a_bc, b_bc, xh2, j)

            gpool = mlp.enter_context(tc.tile_pool(name="gpool", bufs=1))
            g = [gpool.tile([128, T], BF16, tag=f"g{i}", name=f"g{i}") for i in range(Dff // 128)]

            for mt in range(Dff // 128):
                wt = wpool.tile([128, CT, 128], F32R, tag="w", name="w")
                nc.sync.dma_start(
                    wt[:], w_f1.rearrange("(kt p) m -> p kt m", p=128)[:, :, mt * 128:(mt + 1) * 128])
                for j in range(nch):
                    ps = psum([128, chunk], "ps1")
                    for i in range(CT):
                        nc.tensor.matmul(
                            ps[:], wt[:, i, :], xh2[i][:, j * chunk:(j + 1) * chunk],
                            start=(i == 0), stop=(i == CT - 1))
                    nc.scalar.activation(
                        out=g[mt][:, j * chunk:(j + 1) * chunk], in_=ps[:],
                        func=AF.Gelu, bias=bf1_sb[:, mt:mt + 1])
            with tc.tile_pool(name="ostage", bufs=2) as osp:
                for mt in range(CT):
                    wt = wpool.tile([128, Dff // 128, 128], BF16, tag="w", name="w")
                    nc.sync.dma_start(
                        wt[:], w_f2.rearrange("(kt p) m -> p kt m", p=128)[:, :, mt * 128:(mt + 1) * 128])
                    for j in range(nch):
                        ps = psum([128, chunk], "ps2")
                        for i in range(Dff // 128):
                            nc.tensor.matmul(ps[:], wt[:, i, :], g[i][:, j * chunk:(j + 1) * chunk],
                                             start=(i == 0), stop=(i == Dff // 128 - 1))
                        ot = osp.tile([128, chunk], F32, tag="ot", name="ot")
                        nc.vector.tensor_add(ot[:], ps[:], x2[mt][:, j * chunk:(j + 1) * chunk])
                        nc.vector.tensor_scalar_add(ot[:], ot[:], bf2_sb[:, mt:mt + 1])
                        nc.sync.dma_start(
                            out_fm[mt * 128:(mt + 1) * 128, j * chunk:(j + 1) * chunk], ot[:])

    nc.compile()
    return nc


_NC_CACHE = {}


def _get_nc(bl=BL):
    if bl not in _NC_CACHE:
        _NC_CACHE[bl] = build_nc(bl)
    return _NC_CACHE[bl]


def _host_prep(x, prompt, ln1_w, ln1_b, qkv_w, qkv_b, proj_w, proj_b,
               ln2_w, ln2_b, fc1_w, fc1_b, fc2_w, fc2_b, bl=BL, ncores=NCORES):
    import ml_dtypes
    f8 = np.float64
    ln1_w, ln1_b = f8(ln1_w), f8(ln1_b)
    ln2_w, ln2_b = f8(ln2_w), f8(ln2_b)
    qkv_w8, fc1_w8 = f8(qkv_w), f8(fc1_w)

    w_qk = np.ascontiguousarray((qkv_w8[:2 * C] * ln1_w).T.astype(np.float32))
    b_qk = (f8(qkv_b[:2 * C]) + qkv_w8[:2 * C] @ ln1_b).astype(np.float32).reshape(12, 128).T.copy()
    w_v = np.ascontiguousarray((qkv_w8[2 * C:] * ln1_w).T.astype(np.float32))
    b_v = (f8(qkv_b[2 * C:]) + qkv_w8[2 * C:] @ ln1_b).astype(np.float32).reshape(6, 128).T.copy()
    w_pr = np.ascontiguousarray(np.float32(proj_w).T.astype(ml_dtypes.bfloat16))
    b_pr = np.float32(proj_b).reshape(6, 128).T.copy()
    w_f1 = np.ascontiguousarray((fc1_w8 * ln2_w).T.astype(np.float32))
    b_f1 = (f8(fc1_b) + fc1_w8 @ ln2_b).astype(np.float32).reshape(24, 128).T.copy()
    w_f2 = np.ascontiguousarray(np.float32(fc2_w).T.astype(ml_dtypes.bfloat16))
    b_f2 = np.float32(fc2_b).reshape(6, 128).T.copy()

    shared = dict(w_qk=w_qk, b_qk=b_qk, w_v=w_v, b_v=b_v, w_pr=w_pr, b_pr=b_pr,
                  w_f1=w_f1, b_f1=b_f1, w_f2=w_f2, b_f2=b_f2)

    x = np.float32(x).reshape(ncores, bl, N, C)
    prompt = np.float32(prompt).reshape(ncores, bl, P, 2, H, Dh)
    in_maps = []
    for c in range(ncores):
        x_fm = np.ascontiguousarray(x[c].reshape(bl * N, C).T)
        kpc = prompt[c, :, :, 0]                         # [bl, P, H, Dh]
        kpa = np.ascontiguousarray(
            kpc.reshape(bl, P, CT, 128).transpose(2, 3, 0, 1)).astype(ml_dtypes.bfloat16)
        vpc = np.ascontiguousarray(prompt[c, :, :, 1]).astype(ml_dtypes.bfloat16)
        in_maps.append(dict(x_fm=x_fm, kp=kpa, vp=vpc, **shared))
    return in_maps


def run_sharded(inputs, bl=BL, ncores=NCORES, **spmd_kwargs):
    in_maps = _host_prep(**inputs, bl=bl, ncores=ncores)
    nc = _get_nc(bl)
    res = bass_utils.run_bass_kernel_spmd(nc, in_maps, core_ids=list(range(ncores)), **spmd_kwargs)
    outs = [r["out_fm"].T.reshape(bl, N, C) for r in res.results]
    return np.concatenate(outs, axis=0).astype(np.float32), res


def kernel(**inputs):
    out, _ = run_sharded(inputs, bl=BL, ncores=NCORES)
    return out



# revision 44
# speedup vs baseline: 1.3137x; 1.3137x over previous
"""Trainium2 Bass kernel for a ViT-style transformer block (B=64,N=197,C=768,H=12,P=20).

Data-parallel over batch across 8 NeuronCores (8 images/core). Per core:
- fp8e4 DoubleRow matmuls (256-deep contraction, 0.5 cyc/row) for qkv/proj/
  fc1/fc2/attention-scores/AV. fc1/fc2 add an unscaled fp8 residual weight
  pass, and fc1 a third pass on the fp8 residual of xh2, recovering ~fp16
  GEMM accuracy at fp8 speed.
- single software pipeline over 4 two-image chunks: LN1 -> qkv GEMMs ->
  attention (2 images) -> proj(+residual via identity matmul) -> LN2 ->
  fc1+gelu, with fc2 as the tail. Engine assignment balances DVE/ACT/GPSIMD
  under the PE roofline.
- LN: ones(1/C)-matmul column sums, rsqrt as Exp(-0.5*Ln(var+eps)) so the
  ACT table never leaves the exp set; gpsimd partition broadcasts; bf16
  residuals/activations (DVE 4x mode), fp8 casts of xh2/rx2 on gpsimd.
- attention: kv-major scores via DoubleRow over host-permuted paired q/k
  rows; denominator from a ones-column in V (zeroed on kv padding); one
  pair-batched reciprocal per 2 heads into partition 0; gpsimd broadcast;
  normalize fused into the o psum->sbuf write; score/exp software-pipelined
  one head ahead.
"""
import numpy as np
import concourse.bass as bass
import concourse.mybir as mybir
import concourse.tile as tile
from concourse import bacc, bass_utils
from contextlib import ExitStack

F32 = mybir.dt.float32
F32R = mybir.dt.float32r
BF16 = mybir.dt.bfloat16
FP8 = mybir.dt.float8e4
AF = mybir.ActivationFunctionType
DR = mybir.MatmulPerfMode.DoubleRow

B, N, C, H, Dh, P, Dff = 64, 197, 768, 12, 64, 20, 3072
NCORES = 8
BL = B // NCORES
SW = 64.0
KV = 256
NPAD = KV - N
CB = C // 256
GB = Dff // 256
EPS = 1e-6


def build_nc(bl=BL):
    T = bl * N
    nch = max(bl // 2, 1)
    CH = T // nch          # 394
    HC = N                 # 197
    IW = 200               # 4-aligned per-image token stride for fp8 tiles
    VS = 68                # 4-aligned per-head slot in v (Dh + ones + pad)
    sc_exp = Dh ** -0.5

    nc = bacc.Bacc(trn_type="TRN2", target_bir_lowering=False)

    x_fm = nc.dram_tensor("x_fm", [128, 6, T], BF16, kind="ExternalInput")
    xres = nc.dram_tensor("xres", [128, 6, T], BF16, kind="ExternalInput")
    kp8 = nc.dram_tensor("kp8", [128, CB, 2, bl, NPAD], FP8, kind="ExternalInput")
    vp8 = nc.dram_tensor("vp8", [NPAD, bl, H, VS], FP8, kind="ExternalInput")
    w_qk8 = nc.dram_tensor("w_qk8", [128, 12, CB, 2, 128], FP8, kind="ExternalInput")
    w_v8 = nc.dram_tensor("w_v8", [128, CB, 2, C], FP8, kind="ExternalInput")
    w_pr8 = nc.dram_tensor("w_pr8", [128, 6, CB, 2, 128], FP8, kind="ExternalInput")
    w_f1a = nc.dram_tensor("w_f1a", [128, 24, 2, CB, 2, 128], FP8, kind="ExternalInput")
    w_f2a = nc.dram_tensor("w_f2a", [6, 128, 2, GB, 2, 128], FP8, kind="ExternalInput")
    b_qk = nc.dram_tensor("b_qk", [128, 12], F32, kind="ExternalInput")
    b_f1 = nc.dram_tensor("b_f1", [128, 24], F32, kind="ExternalInput")
    b_f2 = nc.dram_tensor("b_f2", [128, 6], F32, kind="ExternalInput")
    ident_d = nc.dram_tensor("ident_d", [128, 128], BF16, kind="ExternalInput")
    out_fm = nc.dram_tensor("out_fm", [128, 6, T], F32, kind="ExternalOutput")
    import os
    _dbg = os.environ.get("KDBG", "")
    dbg_x2 = nc.dram_tensor("dbg_x2", [128, 6, T], BF16, kind="ExternalOutput") \
        if _dbg else None
    dbg_xh2 = nc.dram_tensor("dbg_xh2", [128, CB, 2, bl, 200], FP8,
                             kind="ExternalOutput") if _dbg else None
    dbg_rx2 = nc.dram_tensor("dbg_rx2", [128, CB, 2, bl, 200], FP8,
                             kind="ExternalOutput") if _dbg else None
    dbg_g = nc.dram_tensor("dbg_g", [128, GB, 2, bl, 200], FP8,
                           kind="ExternalOutput") if _dbg else None
    dbg_o = nc.dram_tensor("dbg_o", [128, CB, 2, 2, 200], FP8,
                           kind="ExternalOutput") if _dbg else None
    dbg_q = nc.dram_tensor("dbg_q", [128, CB, 2, 2, 200], FP8,
                           kind="ExternalOutput") if _dbg else None
    dbg_k = nc.dram_tensor("dbg_k", [128, CB, 2, 2, KV], FP8,
                           kind="ExternalOutput") if _dbg else None
    dbg_v = nc.dram_tensor("dbg_v", [128, 2, 2, H, VS], FP8,
                           kind="ExternalOutput") if _dbg else None
    dbg_xh = nc.dram_tensor("dbg_xh", [128, CB, 2, 2, 200], FP8,
                            kind="ExternalOutput") if _dbg else None
    dbg_ln = nc.dram_tensor("dbg_ln", [1, 4, 394], F32,
                            kind="ExternalOutput") if _dbg else None
    dbg_ab = nc.dram_tensor("dbg_ab", [128, 2, 394], BF16,
                            kind="ExternalOutput") if _dbg else None

    with tile.TileContext(nc) as tc, ExitStack() as top:
        top.enter_context(nc.allow_low_precision(reason="fp8/bf16 kernel by design"))

        # ---- early x chunk DMAs go first in the queue ----
        xload = top.enter_context(tc.tile_pool(name="xload", bufs=2))
        xts = {}

        def load_x(j):
            xt = xload.tile([128, 6, CH], BF16, tag="x", name="xt")
            nc.sync.dma_start(xt[:], x_fm[:, :, j * CH:(j + 1) * CH])
            xts[j] = xt

        load_x(0)

        consts = top.enter_context(tc.tile_pool(name="consts", bufs=1))
        onesC = consts.tile([128, 1], BF16)
        nc.vector.memset(onesC[:], 1.0 / C)
        eps1 = consts.tile([1, 1], F32)
        nc.vector.memset(eps1[:], EPS)
        eps2 = consts.tile([1, 1], F32)
        nc.vector.memset(eps2[:], EPS * SW * SW)
        bqk_sb = consts.tile([128, 12], F32)
        nc.sync.dma_start(bqk_sb[:], b_qk[:])
        bf1_sb = consts.tile([128, 24], F32)
        nc.sync.dma_start(bf1_sb[:], b_f1[:])
        bf2_sb = consts.tile([128, 6], F32)
        nc.sync.dma_start(bf2_sb[:], b_f2[:])
        ident = consts.tile([128, 128], BF16)
        nc.sync.dma_start(ident[:], ident_d[:])

        wres = top.enter_context(tc.tile_pool(name="wres", bufs=1))
        wqk = wres.tile([128, 12, CB, 2, 128], FP8)
        nc.sync.dma_start(wqk[:], w_qk8[:])
        wv_sb = wres.tile([128, CB, 2, C], FP8)
        nc.sync.dma_start(wv_sb[:], w_v8[:])
        wpr = wres.tile([128, 6, CB, 2, 128], FP8)
        nc.sync.dma_start(wpr[:], w_pr8[:])

        ps = top.enter_context(tc.tile_pool(name="ps", bufs=2, space="PSUM"))

        main = top.enter_context(tc.tile_pool(name="main", bufs=1))
        x2_sb = main.tile([128, 6, T], BF16)
        xh2 = main.tile([128, CB, 2, bl, IW], FP8)
        rx2 = main.tile([128, CB, 2, bl, IW], FP8)
        g = main.tile([128, GB, 2, bl, IW], FP8)
        chp = top.enter_context(tc.tile_pool(name="chp", bufs=2))

        load_x(1)

        lnp = top.enter_context(tc.tile_pool(name="lnp", bufs=1))
        ap = top.enter_context(tc.tile_pool(name="attn", bufs=3))
        xrp = top.enter_context(tc.tile_pool(name="xrp", bufs=1))

        def ln_stats(src_of, jtag):
            # two bank-aligned rows (512 f32 = one psum bank each)
            st = ps.tile([1, 2, 512], F32, tag="st", bufs=1, name="st")
            for i in range(6):
                nc.tensor.matmul(st[:, 0, 0:CH], onesC[:], src_of(i),
                                 start=(i == 0), stop=(i == 5))
            for i in range(6):
                sq = lnp.tile([128, CH], BF16, tag="sq", name="sq")
                nc.vector.tensor_mul(sq[:], src_of(i), src_of(i))
                nc.tensor.matmul(st[:, 1, 0:CH], onesC[:], sq[:],
                                 start=(i == 0), stop=(i == 5))
            mu_bf = lnp.tile([1, CH], BF16, tag="mu", name="mu_bf")
            nc.vector.tensor_copy(mu_bf[:], st[:, 0, 0:CH])
            mu2 = lnp.tile([1, CH], F32, tag="mu2", name="mu2")
            nc.vector.tensor_mul(mu2[:], mu_bf[:], mu_bf[:])
            var = lnp.tile([1, CH], F32, tag="var", name="var")
            nc.vector.tensor_sub(var[:], st[:, 1, 0:CH], mu2[:])
            return mu_bf, var

        def ln_finish(mu_bf, var, eps_sb):
            sd = lnp.tile([1, CH], F32, tag="sd", name="sd")
            nc.scalar.activation(out=sd[:], in_=var[:], func=AF.Sqrt, bias=eps_sb[:])
            rs = lnp.tile([1, CH], F32, tag="rs", name="rs")
            nc.vector.reciprocal_approx_fast(out=rs[:], in_=sd[:])
            rs_bf = lnp.tile([1, CH], BF16, tag="rsb", name="rs_bf")
            nc.vector.tensor_copy(rs_bf[:], rs[:])
            murs = lnp.tile([1, CH], BF16, tag="mursb", name="murs")
            nc.vector.tensor_mul(murs[:], mu_bf[:], rs_bf[:])
            a_bc = lnp.tile([128, CH], BF16, tag="a_bc", name="a_bc")
            nc.gpsimd.partition_broadcast(a_bc[:], rs_bf[:])
            b_bc = lnp.tile([128, CH], BF16, tag="b_bc", name="b_bc")
            nc.gpsimd.partition_broadcast(b_bc[:], murs[:])
            return a_bc, b_bc

        def emit_s_exp(k_sb, q_sb, li, h):
            hp, hq = h // 4, h % 4
            hsl = slice(hq * 32, (hq + 1) * 32)
            s_ps = ps.tile([128, 2, N], F32, tag="s", bufs=2, name="s_ps")
            for kb in range(2):
                nc.tensor.matmul(
                    s_ps[:, kb, :],
                    k_sb[hsl, hp, :, li, kb * 128:(kb + 1) * 128],
                    q_sb[hsl, hp, :, li, 0:N],
                    start=True, stop=True, perf_mode=DR,
                    tile_position=(hq * 32, 0))
            e_t = ap.tile([128, 2, IW], FP8, tag="e", name="e_t")
            nc.scalar.activation(out=e_t[:, :, 0:N], in_=s_ps[:], func=AF.Exp,
                                 scale=sc_exp)
            return e_t

        def emit_attention(k_sb, q_sb, v_sb, o_fm, li, extra_work=None):
            e_next = emit_s_exp(k_sb, q_sb, li, 0)
            av = None
            isl = slice(li * N, (li + 1) * N)
            for h in range(H):
                e_t = e_next
                if h % 2 == 0:
                    av = ps.tile([Dh + 1, 2, N], F32, tag="av", bufs=2, name="av")
                if h < H - 1:
                    e_next = emit_s_exp(k_sb, q_sb, li, h + 1)
                nc.tensor.matmul(
                    av[:, h % 2, :], v_sb[:, li, :, h, 0:Dh + 1],
                    e_t[:, :, 0:N], start=True, stop=True, perf_mode=DR)
                if extra_work is not None:
                    next(extra_work, None)
                if h % 2 == 1:
                    rt = ap.tile([1, 2, N], BF16, tag="rt", name="rt")
                    nc.vector.reciprocal(rt[:], av[Dh:Dh + 1, :, :])
                    for u in range(2):
                        hu = h - 1 + u
                        rb = ap.tile([64, N], BF16, tag="rb", name="rb")
                        nc.gpsimd.partition_broadcast(rb[:], rt[0:1, u, :])
                        poff, cb2, jd2 = (hu % 2) * 64, hu // 4, (hu // 2) % 2
                        nc.vector.tensor_mul(
                            o_fm[poff:poff + 64, cb2, jd2, li, 0:N],
                            av[0:Dh, u, :], rb[:])

        def fc1_gen(j):
            """Yields after each fc1 mt-group of chunk j (24 yields)."""
            jsl = slice(j * CH, (j + 1) * CH)
            for mt in range(24):
                p1 = ps.tile([128, CH], F32, tag="mm", name="ps1")
                for half in range(2):
                    im_g = 2 * j + half
                    hps = p1[:, half * HC:half * HC + HC]
                    k = 0
                    for wi, mvt in ((0, xh2), (1, xh2), (0, rx2)):
                        for cb in range(CB):
                            nc.tensor.matmul(
                                hps, wf1[:, mt, wi, cb, :, :],
                                mvt[:, cb, :, im_g, 0:HC],
                                start=(k == 0), stop=(k == 8), perf_mode=DR)
                            k += 1
                nc.scalar.activation(
                    out=g[:, mt // 2, mt % 2, 2 * j:2 * j + 2, 0:HC], in_=p1[:],
                    func=AF.Gelu, bias=bf1_sb[:, mt:mt + 1], scale=1.0 / SW)
                yield mt

        # ---------------- main pipeline over chunks ----------------
        for j in range(nch):
            jsl = slice(j * CH, (j + 1) * CH)
            xt = xts[j]
            if j + 1 < nch and j >= 1:
                load_x(j + 1)
            xh8 = chp.tile([128, CB, 2, 2, IW], FP8, tag="xh8", name="xh8")
            q_sb = chp.tile([128, CB, 2, 2, IW], FP8, tag="q_sb", name="q_sb")
            k_sb = chp.tile([128, CB, 2, 2, KV], FP8, tag="k_sb", name="k_sb")
            v_sb = chp.tile([128, 2, 2, H, VS], FP8, tag="v_sb", name="v_sb")
            o_fm = chp.tile([128, CB, 2, 2, IW], FP8, tag="o_fm", name="o_fm")
            nc.vector.memset(v_sb[:, :, :, :, Dh:Dh + 1], 1.0)
            nc.sync.dma_start(k_sb[:, :, :, :, N:KV],
                              kp8[:, :, :, 2 * j:2 * j + 2, :])
            nc.sync.dma_start(v_sb[N - 128:128, :, 1, :, :],
                              vp8[:, 2 * j:2 * j + 2])
            mu_bf, var = ln_stats(lambda i: xt[:, i, :], f"a{j}")
            a_bc, b_bc = ln_finish(mu_bf, var, eps1)
            if dbg_ln is not None and j == 0:
                nc.sync.dma_start(dbg_ln[:, 0, :], var[:])
                lnmu = lnp.tile([1, CH], F32, tag="dbgmu", name="lnmu")
                nc.vector.tensor_copy(lnmu[:], mu_bf[:])
                nc.sync.dma_start(dbg_ln[:, 1, :], lnmu[:])
                nc.sync.dma_start(dbg_ab[:, 0, :], a_bc[:])
                nc.sync.dma_start(dbg_ab[:, 1, :], b_bc[:])
            for i in range(6):
                t = lnp.tile([128, CH], BF16, tag="ap", name="ap")
                nc.vector.tensor_mul(t[:], xt[:, i, :], a_bc[:])
                nc.vector.tensor_sub(xh8[:, i // 2, i % 2, :, 0:HC], t[:], b_bc[:])

            if j == 0:
                wf1 = wres.tile([128, 24, 2, CB, 2, 128], FP8)
                nc.sync.dma_start(wf1[:], w_f1a[:])

            for mt in range(12):
                pq = ps.tile([128, CH], F32, tag="mm", name="psqk")
                for half in range(2):
                    for cb in range(CB):
                        nc.tensor.matmul(
                            pq[:, half * HC:half * HC + HC],
                            wqk[:, mt, cb, :, :], xh8[:, cb, :, half, 0:HC],
                            start=(cb == 0), stop=(cb == CB - 1), perf_mode=DR)
                hp, jd = (mt % 6) // 2, mt % 2
                if mt < 6:
                    nc.vector.tensor_scalar(
                        q_sb[:, hp, jd, :, 0:HC], pq[:], 1.0 / SW,
                        bqk_sb[:, mt:mt + 1], mybir.AluOpType.mult,
                        mybir.AluOpType.add)
                else:
                    nc.scalar.activation(
                        out=k_sb[:, hp, jd, :, 0:N], in_=pq[:],
                        func=AF.Identity, scale=1.0 / SW,
                        bias=bqk_sb[:, mt:mt + 1])

            for v_im in range(2):
                for pt, (toff, tsz) in enumerate([(0, 128), (128, N - 128)]):
                    stat = [xh8[:, cb, :, v_im, toff:toff + tsz]
                            for cb in range(CB)]
                    p2 = ps.tile([128, 2, 256], F32, tag="mm", name="psv2")
                    for vc in range(2):
                        for cb in range(CB):
                            nc.tensor.matmul(
                                p2[:tsz, vc, :], stat[cb],
                                wv_sb[:, cb, :, vc * 256:(vc + 1) * 256],
                                start=(cb == 0), stop=(cb == CB - 1), perf_mode=DR)
                    nc.scalar.activation(
                        out=v_sb[0:tsz, v_im, pt, 0:8, 0:Dh], in_=p2[:tsz, :, :],
                        func=AF.Copy, scale=1.0 / SW)
                    p1 = ps.tile([128, 256], F32, tag="mm", name="psv1")
                    for cb in range(CB):
                        nc.tensor.matmul(
                            p1[:tsz, :], stat[cb], wv_sb[:, cb, :, 512:768],
                            start=(cb == 0), stop=(cb == CB - 1), perf_mode=DR)
                    nc.scalar.activation(
                        out=v_sb[0:tsz, v_im, pt, 8:12, 0:Dh], in_=p1[:tsz, :],
                        func=AF.Copy, scale=1.0 / SW)

            emit_attention(k_sb, q_sb, v_sb, o_fm, 0)
            emit_attention(k_sb, q_sb, v_sb, o_fm, 1)
            if dbg_o is not None and j == 0:
                nc.sync.dma_start(dbg_xh[:], xh8[:])
                nc.sync.dma_start(dbg_o[:], o_fm[:])
                nc.sync.dma_start(dbg_q[:], q_sb[:])
                nc.sync.dma_start(dbg_k[:], k_sb[:])
                nc.sync.dma_start(dbg_v[:], v_sb[:])

            # proj + residual (identity matmul) + LN2 stats/finish/apply
            xr = xrp.tile([128, 6, CH], BF16, tag="xr", name="xr")
            nc.sync.dma_start(xr[:], xres[:, :, jsl])
            for mt in range(6):
                pp = ps.tile([128, CH], F32, tag="mm", name="pspr")
                for half in range(2):
                    hps = pp[:, half * HC:half * HC + HC]
                    for cb in range(CB):
                        nc.tensor.matmul(
                            hps, wpr[:, mt, cb, :, :], o_fm[:, cb, :, half, 0:HC],
                            start=(cb == 0), stop=False, perf_mode=DR)
                    nc.tensor.matmul(
                        hps, ident[:], xr[:, mt, half * HC:half * HC + HC],
                        start=False, stop=True, skip_group_check=True)
                nc.scalar.activation(
                    out=x2_sb[:, mt, jsl], in_=pp[:], func=AF.Identity)
            mu_bf, var = ln_stats(lambda i: x2_sb[:, i, jsl], f"b{j}")
            a_bc, b_bc = ln_finish(mu_bf, var, eps2)
            for i in range(6):
                t = lnp.tile([128, CH], BF16, tag="ap", name="ap")
                nc.vector.tensor_mul(t[:], x2_sb[:, i, jsl], a_bc[:])
                xbf = lnp.tile([128, CH], BF16, tag="xbf", name="xbf")
                nc.vector.tensor_sub(xbf[:], t[:], b_bc[:])
                nc.vector.tensor_copy(xh2[:, i // 2, i % 2, 2 * j:2 * j + 2, 0:HC],
                                      xbf[:])
                nc.gpsimd.tensor_sub(rx2[:, i // 2, i % 2, 2 * j:2 * j + 2, 0:HC],
                                     xbf[:],
                                     xh2[:, i // 2, i % 2, 2 * j:2 * j + 2, 0:HC])

            if j < nch - 1:
                for _ in fc1_gen(j):
                    pass

        if dbg_x2 is not None:
            nc.sync.dma_start(dbg_x2[:], x2_sb[:])
            nc.sync.dma_start(dbg_xh2[:], xh2[:])
            nc.sync.dma_start(dbg_rx2[:], rx2[:])
            nc.sync.dma_start(dbg_g[:], g[:])

        # ------- tail: fc1(last chunk) interleaved with fc2 cols 0..nch-2 -------
        fc1_last = fc1_gen(nch - 1)
        with ExitStack() as ph5:
            wpool = ph5.enter_context(tc.tile_pool(name="wpool", bufs=2))
            opool = ph5.enter_context(tc.tile_pool(name="opool", bufs=2))

            def fc2_group(mt, j, w2):
                jsl = slice(j * CH, (j + 1) * CH)
                p2 = ps.tile([128, CH], F32, tag="mm", name="ps2")
                for half in range(2):
                    im_g = 2 * j + half
                    hsl = slice(j * CH + half * HC, j * CH + half * HC + HC)
                    hps = p2[:, half * HC:half * HC + HC]
                    k = 0
                    for wi in range(2):
                        for gb in range(GB):
                            nc.tensor.matmul(
                                hps, w2[:, wi, gb, :, :], g[:, gb, :, im_g, 0:HC],
                                start=(k == 0), stop=False, perf_mode=DR)
                            k += 1
                    nc.tensor.matmul(
                        hps, ident[:], x2_sb[:, mt, hsl],
                        start=False, stop=True, skip_group_check=True)
                ot = opool.tile([128, CH], F32, tag="ot", name="ot")
                nc.vector.tensor_scalar(ot[:], p2[:], 1.0 / SW,
                                        bf2_sb[:, mt:mt + 1],
                                        mybir.AluOpType.mult,
                                        mybir.AluOpType.add)
                nc.sync.dma_start(out_fm[:, mt, jsl], ot[:])

            slot = 0
            for mt in range(6):
                w2 = wpool.tile([128, 2, GB, 2, 128], FP8, tag="w2", name="w2")
                nc.sync.dma_start(w2[:], w_f2a[mt])
                for j in range(nch - 1):
                    fc2_group(mt, j, w2)
                    next(fc1_last, None)
                    if slot % 3 == 0:
                        next(fc1_last, None)
                    slot += 1
            for _ in fc1_last:
                pass
            for mt in range(6):
                w2 = wpool.tile([128, 2, GB, 2, 128], FP8, tag="w2", name="w2")
                nc.sync.dma_start(w2[:], w_f2a[mt])
                fc2_group(mt, nch - 1, w2)

    nc.compile()
    return nc


_NC_CACHE = {}


def _get_nc(bl=BL):
    if bl not in _NC_CACHE:
        _NC_CACHE[bl] = build_nc(bl)
    return _NC_CACHE[bl]


def _q8(a):
    import ml_dtypes
    return np.asarray(a, np.float32).astype(ml_dtypes.float8_e4m3)


def _qb(a):
    import ml_dtypes
    return np.asarray(a, np.float32).astype(ml_dtypes.bfloat16)


def _stationary(w, nmt):
    o, kc = w.shape
    nb = kc // 256
    return np.ascontiguousarray(
        w.reshape(nmt, 128, nb, 2, 128).transpose(0, 4, 2, 3, 1))


def _host_prep(x, prompt, ln1_w, ln1_b, qkv_w, qkv_b, proj_w, proj_b,
               ln2_w, ln2_b, fc1_w, fc1_b, fc2_w, fc2_b, bl=BL, ncores=NCORES):
    import ml_dtypes
    f8 = np.float64
    w_qk = (f8(qkv_w[:2 * C]) * f8(ln1_w)).astype(np.float32)
    b_qkf = (f8(qkv_b[:2 * C]) + f8(qkv_w[:2 * C]) @ f8(ln1_b)).astype(np.float32)
    w_v = (f8(qkv_w[2 * C:]) * f8(ln1_w)).astype(np.float32)
    b_v = (f8(qkv_b[2 * C:]) + f8(qkv_w[2 * C:]) @ f8(ln1_b)).astype(np.float32)
    b_pr = (f8(proj_b) + f8(proj_w) @ f8(b_v)).astype(np.float32)
    w_f1 = (f8(fc1_w) * f8(ln2_w)).astype(np.float32)
    b_f1f = (f8(fc1_b) + f8(fc1_w) @ f8(ln2_b)).astype(np.float32)

    perm = np.zeros(C, dtype=np.int64)
    for mt6 in range(6):
        hp, jd = mt6 // 2, mt6 % 2
        for pr in range(128):
            hq, dl = pr // 32, pr % 32
            perm[mt6 * 128 + pr] = (hp * 4 + hq) * 64 + jd * 32 + dl

    wq = _stationary(_q8(w_qk[:C][perm] * SW), 6)
    wk = _stationary(_q8(w_qk[C:][perm] * SW), 6)
    w_qk8 = np.ascontiguousarray(
        np.concatenate([wq, wk], axis=0).transpose(1, 0, 2, 3, 4))
    b_qk_p = np.concatenate([b_qkf[:C][perm].reshape(6, 128),
                             b_qkf[C:][perm].reshape(6, 128)], axis=0).T.copy()

    wv8 = _q8(w_v * SW)
    w_v8 = np.ascontiguousarray(wv8.T.reshape(CB, 2, 128, C).transpose(2, 0, 1, 3))

    w_pr8 = np.ascontiguousarray(
        _stationary(_q8(np.float32(proj_w) * SW), 6).transpose(1, 0, 2, 3, 4))

    wf1s = np.float32(w_f1) * SW
    wf18 = _q8(wf1s)
    wf1r = _q8(wf1s - np.float32(wf18))
    w_f1a = np.stack([_stationary(wf18, 24), _stationary(wf1r, 24)], axis=1)
    w_f1a = np.ascontiguousarray(w_f1a.transpose(2, 0, 1, 3, 4, 5))

    wf2s = np.float32(fc2_w) * SW
    wf28 = _q8(wf2s)
    wf2r = _q8(wf2s - np.float32(wf28))
    w_f2a = np.stack([_stationary(wf28, 6), _stationary(wf2r, 6)], axis=1)
    w_f2a = np.ascontiguousarray(w_f2a.transpose(0, 2, 1, 3, 4, 5))

    b_f1a = np.float32(b_f1f).reshape(24, 128).T.copy()
    b_f2a = np.float32(fc2_b).reshape(6, 128).T.copy()
    ident = np.eye(128, dtype=ml_dtypes.bfloat16)

    shared = dict(w_qk8=w_qk8, w_v8=w_v8, w_pr8=w_pr8, w_f1a=w_f1a, w_f2a=w_f2a,
                  b_qk=b_qk_p, b_f1=b_f1a, b_f2=b_f2a, ident_d=ident)

    T = bl * N
    x = np.float32(x).reshape(ncores, bl, N, C)
    prompt = np.float32(prompt).reshape(ncores, bl, P, 2, H, Dh)
    in_maps = []
    for c in range(ncores):
        xc = x[c].reshape(T, C)
        x_fmc = np.ascontiguousarray(
            _qb(xc.T).reshape(6, 128, T).transpose(1, 0, 2))
        xresc = np.ascontiguousarray(
            _qb((xc + b_pr).T * SW).reshape(6, 128, T).transpose(1, 0, 2))
        pk = np.float32(_q8(prompt[c, :, :, 0]))
        pk = pk.reshape(bl, P, CB, 4, 2, 32).transpose(3, 5, 2, 4, 0, 1)
        kp8 = np.zeros((128, CB, 2, bl, NPAD), dtype=ml_dtypes.float8_e4m3)
        kp8[:, :, :, :, :P] = _q8(pk.reshape(128, CB, 2, bl, P))
        vp8 = np.zeros((NPAD, bl, H, 68), dtype=ml_dtypes.float8_e4m3)
        vp8[:P, :, :, :Dh] = _q8(prompt[c, :, :, 1].transpose(1, 0, 2, 3))
        vp8[:P, :, :, Dh] = 1.0
        in_maps.append(dict(x_fm=x_fmc, xres=xresc, kp8=kp8, vp8=vp8, **shared))
    return in_maps


def run_sharded(inputs, bl=BL, ncores=NCORES, **spmd_kwargs):
    in_maps = _host_prep(**inputs, bl=bl, ncores=ncores)
    nc = _get_nc(bl)
    res = bass_utils.run_bass_kernel_spmd(nc, in_maps, core_ids=list(range(ncores)),
                                          **spmd_kwargs)
    T = bl * N
    outs = [r["out_fm"].transpose(1, 0, 2).reshape(C, T).T.reshape(bl, N, C)
            for r in res.results]
    return np.concatenate(outs, axis=0).astype(np.float32), res


def kernel(**inputs):
    out, _ = run_sharded(inputs, bl=BL, ncores=NCORES)
    return out


# revision 47
# speedup vs baseline: 1.3236x; 1.0075x over previous
"""Trainium2 Bass kernel for a ViT-style transformer block (B=64,N=197,C=768,H=12,P=20).

Data-parallel over batch across 8 NeuronCores (8 images/core). Per core:
- fp8e4 DoubleRow matmuls (256-deep contraction, 0.5 cyc/row) for qkv/proj/
  fc1/fc2/attention-scores/AV. fc1/fc2 add an unscaled fp8 residual weight
  pass, and fc1 a third pass on the fp8 residual of xh2, recovering ~fp16
  GEMM accuracy at fp8 speed.
- single software pipeline over 4 two-image chunks: LN1 -> qkv GEMMs ->
  attention (2 images) -> proj(+residual via identity matmul) -> LN2 ->
  fc1+gelu, with fc2 as the tail. Engine assignment balances DVE/ACT/GPSIMD
  under the PE roofline.
- LN: ones(1/C)-matmul column sums, rsqrt as Exp(-0.5*Ln(var+eps)) so the
  ACT table never leaves the exp set; gpsimd partition broadcasts; bf16
  residuals/activations (DVE 4x mode), fp8 casts of xh2/rx2 on gpsimd.
- attention: kv-major scores via DoubleRow over host-permuted paired q/k
  rows; denominator from a ones-column in V (zeroed on kv padding); one
  pair-batched reciprocal per 2 heads into partition 0; gpsimd broadcast;
  normalize fused into the o psum->sbuf write; score/exp software-pipelined
  one head ahead.
"""
import numpy as np
import concourse.bass as bass
import concourse.mybir as mybir
import concourse.tile as tile
from concourse import bacc, bass_utils
from contextlib import ExitStack

F32 = mybir.dt.float32
F32R = mybir.dt.float32r
BF16 = mybir.dt.bfloat16
FP8 = mybir.dt.float8e4
AF = mybir.ActivationFunctionType
DR = mybir.MatmulPerfMode.DoubleRow

B, N, C, H, Dh, P, Dff = 64, 197, 768, 12, 64, 20, 3072
NCORES = 8
BL = B // NCORES
SW = 64.0
KV = 256
NPAD = KV - N
CB = C // 256
GB = Dff // 256
EPS = 1e-6


def _merge_lnexp_table():
    """Make natural_log_exp_and_others the only table offering exp/ln by
    emptying the competing sets. Positions (= act_func_set_ids used by
    walrus) are preserved; only selection changes."""
    import concourse.bacc as bacc_mod
    import concourse.hw_specs as hw_specs_mod
    orig = hw_specs_mod.get_activation_tables
    if getattr(bacc_mod.get_activation_tables, "_lnexp_merged", False):
        return

    def filtered(arch):
        t = orig(arch)
        out = {}
        for k, funcs in t.items():
            if k != "natural_log_exp_and_others" and any(
                    f.name in ("Exp", "Ln") for f in funcs):
                out[k] = set()
            else:
                out[k] = funcs
        return out

    filtered._lnexp_merged = True
    bacc_mod.get_activation_tables = filtered


def build_nc(bl=BL):
    _merge_lnexp_table()
    T = bl * N
    nch = max(bl // 2, 1)
    CH = T // nch          # 394
    HC = N                 # 197
    IW = 200               # 4-aligned per-image token stride for fp8 tiles
    VS = 68                # 4-aligned per-head slot in v (Dh + ones + pad)
    sc_exp = Dh ** -0.5

    nc = bacc.Bacc(trn_type="TRN2", target_bir_lowering=False)

    x_fm = nc.dram_tensor("x_fm", [128, 6, T], BF16, kind="ExternalInput")
    xres = nc.dram_tensor("xres", [128, 6, T], BF16, kind="ExternalInput")
    kp8 = nc.dram_tensor("kp8", [128, CB, 2, bl, NPAD], FP8, kind="ExternalInput")
    vp8 = nc.dram_tensor("vp8", [NPAD, bl, H, VS], FP8, kind="ExternalInput")
    w_qk8 = nc.dram_tensor("w_qk8", [128, 12, CB, 2, 128], FP8, kind="ExternalInput")
    w_v8 = nc.dram_tensor("w_v8", [128, CB, 2, C], FP8, kind="ExternalInput")
    w_pr8 = nc.dram_tensor("w_pr8", [128, 6, CB, 2, 128], FP8, kind="ExternalInput")
    w_f1a = nc.dram_tensor("w_f1a", [128, 24, 2, CB, 2, 128], FP8, kind="ExternalInput")
    w_f2a = nc.dram_tensor("w_f2a", [6, 128, 2, GB, 2, 128], FP8, kind="ExternalInput")
    b_qk = nc.dram_tensor("b_qk", [128, 12], F32, kind="ExternalInput")
    b_f1 = nc.dram_tensor("b_f1", [128, 24], F32, kind="ExternalInput")
    b_f2 = nc.dram_tensor("b_f2", [128, 6], F32, kind="ExternalInput")
    ident_d = nc.dram_tensor("ident_d", [128, 128], BF16, kind="ExternalInput")
    out_fm = nc.dram_tensor("out_fm", [128, 6, T], F32, kind="ExternalOutput")
    import os
    _dbg = os.environ.get("KDBG", "")
    dbg_x2 = nc.dram_tensor("dbg_x2", [128, 6, T], BF16, kind="ExternalOutput") \
        if _dbg else None
    dbg_xh2 = nc.dram_tensor("dbg_xh2", [128, CB, 2, bl, 200], FP8,
                             kind="ExternalOutput") if _dbg else None
    dbg_rx2 = nc.dram_tensor("dbg_rx2", [128, CB, 2, bl, 200], FP8,
                             kind="ExternalOutput") if _dbg else None
    dbg_g = nc.dram_tensor("dbg_g", [128, GB, 2, bl, 200], FP8,
                           kind="ExternalOutput") if _dbg else None
    dbg_o = nc.dram_tensor("dbg_o", [128, CB, 2, 2, 200], FP8,
                           kind="ExternalOutput") if _dbg else None
    dbg_q = nc.dram_tensor("dbg_q", [128, CB, 2, 2, 200], FP8,
                           kind="ExternalOutput") if _dbg else None
    dbg_k = nc.dram_tensor("dbg_k", [128, CB, 2, 2, KV], FP8,
                           kind="ExternalOutput") if _dbg else None
    dbg_v = nc.dram_tensor("dbg_v", [128, 2, 2, H, VS], FP8,
                           kind="ExternalOutput") if _dbg else None
    dbg_xh = nc.dram_tensor("dbg_xh", [128, CB, 2, 2, 200], FP8,
                            kind="ExternalOutput") if _dbg else None
    dbg_ln = nc.dram_tensor("dbg_ln", [1, 4, 394], F32,
                            kind="ExternalOutput") if _dbg else None
    dbg_ab = nc.dram_tensor("dbg_ab", [128, 2, 394], BF16,
                            kind="ExternalOutput") if _dbg else None

    with tile.TileContext(nc) as tc, ExitStack() as top:
        top.enter_context(nc.allow_low_precision(reason="fp8/bf16 kernel by design"))

        # ---- early x chunk DMAs go first in the queue ----
        xload = top.enter_context(tc.tile_pool(name="xload", bufs=2))
        xts = {}

        def load_x(j):
            xt = xload.tile([128, 6, CH], BF16, tag="x", name="xt")
            nc.sync.dma_start(xt[:], x_fm[:, :, j * CH:(j + 1) * CH])
            xts[j] = xt

        load_x(0)

        consts = top.enter_context(tc.tile_pool(name="consts", bufs=1))
        onesC = consts.tile([128, 1], BF16)
        nc.vector.memset(onesC[:], 1.0 / C)
        eps1 = consts.tile([1, 1], F32)
        nc.vector.memset(eps1[:], EPS)
        eps2 = consts.tile([1, 1], F32)
        nc.vector.memset(eps2[:], EPS * SW * SW)
        bqk_sb = consts.tile([128, 12], F32)
        nc.sync.dma_start(bqk_sb[:], b_qk[:])
        bf1_sb = consts.tile([128, 24], F32)
        nc.sync.dma_start(bf1_sb[:], b_f1[:])
        bf2_sb = consts.tile([128, 6], F32)
        nc.sync.dma_start(bf2_sb[:], b_f2[:])
        ident = consts.tile([128, 128], BF16)
        nc.sync.dma_start(ident[:], ident_d[:])

        wres = top.enter_context(tc.tile_pool(name="wres", bufs=1))
        wqk = wres.tile([128, 12, CB, 2, 128], FP8)
        nc.sync.dma_start(wqk[:], w_qk8[:])
        wv_sb = wres.tile([128, CB, 2, C], FP8)
        nc.sync.dma_start(wv_sb[:], w_v8[:])
        wpr = wres.tile([128, 6, CB, 2, 128], FP8)
        nc.sync.dma_start(wpr[:], w_pr8[:])

        ps = top.enter_context(tc.tile_pool(name="ps", bufs=2, space="PSUM"))

        main = top.enter_context(tc.tile_pool(name="main", bufs=1))
        x2_sb = main.tile([128, 6, T], BF16)
        xh2 = main.tile([128, CB, 2, bl, IW], FP8)
        rx2 = main.tile([128, CB, 2, bl, IW], FP8)
        g = main.tile([128, GB, 2, bl, IW], FP8)
        chp = top.enter_context(tc.tile_pool(name="chp", bufs=2))

        load_x(1)

        lnp = top.enter_context(tc.tile_pool(name="lnp", bufs=1))
        ap = top.enter_context(tc.tile_pool(name="attn", bufs=3))
        xrp = top.enter_context(tc.tile_pool(name="xrp", bufs=1))

        def ln_stats(src_of, jtag):
            # two bank-aligned rows (512 f32 = one psum bank each)
            st = ps.tile([1, 2, 512], F32, tag="st", bufs=1, name="st")
            for i in range(6):
                nc.tensor.matmul(st[:, 0, 0:CH], onesC[:], src_of(i),
                                 start=(i == 0), stop=(i == 5))
            for i in range(6):
                sq = lnp.tile([128, CH], BF16, tag="sq", name="sq")
                nc.vector.tensor_mul(sq[:], src_of(i), src_of(i))
                nc.tensor.matmul(st[:, 1, 0:CH], onesC[:], sq[:],
                                 start=(i == 0), stop=(i == 5))
            mu_bf = lnp.tile([1, CH], BF16, tag="mu", name="mu_bf")
            nc.vector.tensor_copy(mu_bf[:], st[:, 0, 0:CH])
            mu2 = lnp.tile([1, CH], F32, tag="mu2", name="mu2")
            nc.vector.tensor_mul(mu2[:], mu_bf[:], mu_bf[:])
            var = lnp.tile([1, CH], F32, tag="var", name="var")
            nc.vector.tensor_sub(var[:], st[:, 1, 0:CH], mu2[:])
            return mu_bf, var

        def ln_finish(mu_bf, var, eps_sb):
            # rsqrt = Exp(-0.5*Ln(var+eps)); both funcs live in the merged
            # natural_log_exp_and_others table, so no load near the attention
            # exp stream.
            lv = lnp.tile([1, CH], F32, tag="lv", name="lv")
            nc.scalar.activation(out=lv[:], in_=var[:], func=AF.Ln, bias=eps_sb[:])
            rs = lnp.tile([1, CH], F32, tag="rs", name="rs")
            nc.scalar.activation(out=rs[:], in_=lv[:], func=AF.Exp, scale=-0.5)
            rs_bf = lnp.tile([1, CH], BF16, tag="rsb", name="rs_bf")
            nc.vector.tensor_copy(rs_bf[:], rs[:])
            murs = lnp.tile([1, CH], BF16, tag="mursb", name="murs")
            nc.vector.tensor_mul(murs[:], mu_bf[:], rs_bf[:])
            a_bc = lnp.tile([128, CH], BF16, tag="a_bc", name="a_bc")
            nc.gpsimd.partition_broadcast(a_bc[:], rs_bf[:])
            b_bc = lnp.tile([128, CH], BF16, tag="b_bc", name="b_bc")
            nc.gpsimd.partition_broadcast(b_bc[:], murs[:])
            return a_bc, b_bc

        def emit_s_exp(k_sb, q_sb, li, h):
            hp, hq = h // 4, h % 4
            hsl = slice(hq * 32, (hq + 1) * 32)
            s_ps = ps.tile([128, 2, N], F32, tag="s", bufs=2, name="s_ps")
            for kb in range(2):
                nc.tensor.matmul(
                    s_ps[:, kb, :],
                    k_sb[hsl, hp, :, li, kb * 128:(kb + 1) * 128],
                    q_sb[hsl, hp, :, li, 0:N],
                    start=True, stop=True, perf_mode=DR,
                    tile_position=(hq * 32, 0))
            e_t = ap.tile([128, 2, IW], FP8, tag="e", name="e_t")
            nc.scalar.activation(out=e_t[:, :, 0:N], in_=s_ps[:], func=AF.Exp,
                                 scale=sc_exp)
            return e_t

        def emit_attention(k_sb, q_sb, v_sb, o_fm, li, extra_work=None):
            e_next = emit_s_exp(k_sb, q_sb, li, 0)
            av = None
            isl = slice(li * N, (li + 1) * N)
            for h in range(H):
                e_t = e_next
                if h % 2 == 0:
                    av = ps.tile([Dh + 1, 2, N], F32, tag="av", bufs=2, name="av")
                if h < H - 1:
                    e_next = emit_s_exp(k_sb, q_sb, li, h + 1)
                nc.tensor.matmul(
                    av[:, h % 2, :], v_sb[:, li, :, h, 0:Dh + 1],
                    e_t[:, :, 0:N], start=True, stop=True, perf_mode=DR)
                if extra_work is not None:
                    next(extra_work, None)
                if h % 2 == 1:
                    rt = ap.tile([1, 2, N], BF16, tag="rt", name="rt")
                    nc.vector.reciprocal(rt[:], av[Dh:Dh + 1, :, :])
                    for u in range(2):
                        hu = h - 1 + u
                        rb = ap.tile([64, N], BF16, tag="rb", name="rb")
                        nc.gpsimd.partition_broadcast(rb[:], rt[0:1, u, :])
                        poff, cb2, jd2 = (hu % 2) * 64, hu // 4, (hu // 2) % 2
                        nc.vector.tensor_mul(
                            o_fm[poff:poff + 64, cb2, jd2, li, 0:N],
                            av[0:Dh, u, :], rb[:])

        def fc1_gen(j):
            """Yields after each fc1 mt-group of chunk j (24 yields)."""
            jsl = slice(j * CH, (j + 1) * CH)
            for mt in range(24):
                p1 = ps.tile([128, CH], F32, tag="mm", name="ps1")
                for half in range(2):
                    im_g = 2 * j + half
                    hps = p1[:, half * HC:half * HC + HC]
                    k = 0
                    for wi, mvt in ((0, xh2), (1, xh2), (0, rx2)):
                        for cb in range(CB):
                            nc.tensor.matmul(
                                hps, wf1[:, mt, wi, cb, :, :],
                                mvt[:, cb, :, im_g, 0:HC],
                                start=(k == 0), stop=(k == 8), perf_mode=DR)
                            k += 1
                nc.scalar.activation(
                    out=g[:, mt // 2, mt % 2, 2 * j:2 * j + 2, 0:HC], in_=p1[:],
                    func=AF.Gelu, bias=bf1_sb[:, mt:mt + 1], scale=1.0 / SW)
                yield mt

        # ---------------- main pipeline over chunks ----------------
        for j in range(nch):
            jsl = slice(j * CH, (j + 1) * CH)
            xt = xts[j]
            if j + 1 < nch and j >= 1:
                load_x(j + 1)
            xh8 = chp.tile([128, CB, 2, 2, IW], FP8, tag="xh8", name="xh8")
            q_sb = chp.tile([128, CB, 2, 2, IW], FP8, tag="q_sb", name="q_sb")
            k_sb = chp.tile([128, CB, 2, 2, KV], FP8, tag="k_sb", name="k_sb")
            v_sb = chp.tile([128, 2, 2, H, VS], FP8, tag="v_sb", name="v_sb")
            o_fm = chp.tile([128, CB, 2, 2, IW], FP8, tag="o_fm", name="o_fm")
            nc.vector.memset(v_sb[:, :, :, :, Dh:Dh + 1], 1.0)
            nc.sync.dma_start(k_sb[:, :, :, :, N:KV],
                              kp8[:, :, :, 2 * j:2 * j + 2, :])
            nc.sync.dma_start(v_sb[N - 128:128, :, 1, :, :],
                              vp8[:, 2 * j:2 * j + 2])
            mu_bf, var = ln_stats(lambda i: xt[:, i, :], f"a{j}")
            a_bc, b_bc = ln_finish(mu_bf, var, eps1)
            if dbg_ln is not None and j == 0:
                nc.sync.dma_start(dbg_ln[:, 0, :], var[:])
                lnmu = lnp.tile([1, CH], F32, tag="dbgmu", name="lnmu")
                nc.vector.tensor_copy(lnmu[:], mu_bf[:])
                nc.sync.dma_start(dbg_ln[:, 1, :], lnmu[:])
                nc.sync.dma_start(dbg_ab[:, 0, :], a_bc[:])
                nc.sync.dma_start(dbg_ab[:, 1, :], b_bc[:])
            for i in range(6):
                t = lnp.tile([128, CH], BF16, tag="ap", name="ap")
                nc.vector.tensor_mul(t[:], xt[:, i, :], a_bc[:])
                nc.vector.tensor_sub(xh8[:, i // 2, i % 2, :, 0:HC], t[:], b_bc[:])

            if j == 0:
                wf1 = wres.tile([128, 24, 2, CB, 2, 128], FP8)
                nc.sync.dma_start(wf1[:], w_f1a[:])

            for mt in range(12):
                pq = ps.tile([128, CH], F32, tag="mm", name="psqk")
                for half in range(2):
                    for cb in range(CB):
                        nc.tensor.matmul(
                            pq[:, half * HC:half * HC + HC],
                            wqk[:, mt, cb, :, :], xh8[:, cb, :, half, 0:HC],
                            start=(cb == 0), stop=(cb == CB - 1), perf_mode=DR)
                hp, jd = (mt % 6) // 2, mt % 2
                if mt < 6:
                    nc.vector.tensor_scalar(
                        q_sb[:, hp, jd, :, 0:HC], pq[:], 1.0 / SW,
                        bqk_sb[:, mt:mt + 1], mybir.AluOpType.mult,
                        mybir.AluOpType.add)
                else:
                    nc.scalar.activation(
                        out=k_sb[:, hp, jd, :, 0:N], in_=pq[:],
                        func=AF.Identity, scale=1.0 / SW,
                        bias=bqk_sb[:, mt:mt + 1])

            for v_im in range(2):
                for pt, (toff, tsz) in enumerate([(0, 128), (128, N - 128)]):
                    stat = [xh8[:, cb, :, v_im, toff:toff + tsz]
                            for cb in range(CB)]
                    p2 = ps.tile([128, 2, 256], F32, tag="mm", name="psv2")
                    for vc in range(2):
                        for cb in range(CB):
                            nc.tensor.matmul(
                                p2[:tsz, vc, :], stat[cb],
                                wv_sb[:, cb, :, vc * 256:(vc + 1) * 256],
                                start=(cb == 0), stop=(cb == CB - 1), perf_mode=DR)
                    nc.scalar.activation(
                        out=v_sb[0:tsz, v_im, pt, 0:8, 0:Dh], in_=p2[:tsz, :, :],
                        func=AF.Copy, scale=1.0 / SW)
                    p1 = ps.tile([128, 256], F32, tag="mm", name="psv1")
                    for cb in range(CB):
                        nc.tensor.matmul(
                            p1[:tsz, :], stat[cb], wv_sb[:, cb, :, 512:768],
                            start=(cb == 0), stop=(cb == CB - 1), perf_mode=DR)
                    nc.scalar.activation(
                        out=v_sb[0:tsz, v_im, pt, 8:12, 0:Dh], in_=p1[:tsz, :],
                        func=AF.Copy, scale=1.0 / SW)

            emit_attention(k_sb, q_sb, v_sb, o_fm, 0)
            emit_attention(k_sb, q_sb, v_sb, o_fm, 1)
            if dbg_o is not None and j == 0:
                nc.sync.dma_start(dbg_xh[:], xh8[:])
                nc.sync.dma_start(dbg_o[:], o_fm[:])
                nc.sync.dma_start(dbg_q[:], q_sb[:])
                nc.sync.dma_start(dbg_k[:], k_sb[:])
                nc.sync.dma_start(dbg_v[:], v_sb[:])

            # proj + residual (identity matmul) + LN2 stats/finish/apply
            xr = xrp.tile([128, 6, CH], BF16, tag="xr", name="xr")
            nc.sync.dma_start(xr[:], xres[:, :, jsl])
            for mt in range(6):
                pp = ps.tile([128, CH], F32, tag="mm", name="pspr")
                for half in range(2):
                    hps = pp[:, half * HC:half * HC + HC]
                    for cb in range(CB):
                        nc.tensor.matmul(
                            hps, wpr[:, mt, cb, :, :], o_fm[:, cb, :, half, 0:HC],
                            start=(cb == 0), stop=False, perf_mode=DR)
                    nc.tensor.matmul(
                        hps, ident[:], xr[:, mt, half * HC:half * HC + HC],
                        start=False, stop=True, skip_group_check=True)
                nc.scalar.activation(
                    out=x2_sb[:, mt, jsl], in_=pp[:], func=AF.Identity)
            mu_bf, var = ln_stats(lambda i: x2_sb[:, i, jsl], f"b{j}")
            a_bc, b_bc = ln_finish(mu_bf, var, eps2)
            for i in range(6):
                t = lnp.tile([128, CH], BF16, tag="ap", name="ap")
                nc.vector.tensor_mul(t[:], x2_sb[:, i, jsl], a_bc[:])
                xbf = lnp.tile([128, CH], BF16, tag="xbf", name="xbf")
                nc.vector.tensor_sub(xbf[:], t[:], b_bc[:])
                nc.vector.tensor_copy(xh2[:, i // 2, i % 2, 2 * j:2 * j + 2, 0:HC],
                                      xbf[:])
                nc.gpsimd.tensor_sub(rx2[:, i // 2, i % 2, 2 * j:2 * j + 2, 0:HC],
                                     xbf[:],
                                     xh2[:, i // 2, i % 2, 2 * j:2 * j + 2, 0:HC])

            if j < nch - 1:
                for _ in fc1_gen(j):
                    pass

        if dbg_x2 is not None:
            nc.sync.dma_start(dbg_x2[:], x2_sb[:])
            nc.sync.dma_start(dbg_xh2[:], xh2[:])
            nc.sync.dma_start(dbg_rx2[:], rx2[:])
            nc.sync.dma_start(dbg_g[:], g[:])

        # ------- tail: fc1(last chunk) interleaved with fc2 cols 0..nch-2 -------
        fc1_last = fc1_gen(nch - 1)
        with ExitStack() as ph5:
            wpool = ph5.enter_context(tc.tile_pool(name="wpool", bufs=2))
            opool = ph5.enter_context(tc.tile_pool(name="opool", bufs=2))

            def fc2_group(mt, j, w2):
                jsl = slice(j * CH, (j + 1) * CH)
                p2 = ps.tile([128, CH], F32, tag="mm", name="ps2")
                for half in range(2):
                    im_g = 2 * j + half
                    hsl = slice(j * CH + half * HC, j * CH + half * HC + HC)
                    hps = p2[:, half * HC:half * HC + HC]
                    k = 0
                    for wi in range(2):
                        for gb in range(GB):
                            nc.tensor.matmul(
                                hps, w2[:, wi, gb, :, :], g[:, gb, :, im_g, 0:HC],
                                start=(k == 0), stop=False, perf_mode=DR)
                            k += 1
                    nc.tensor.matmul(
                        hps, ident[:], x2_sb[:, mt, hsl],
                        start=False, stop=True, skip_group_check=True)
                ot = opool.tile([128, CH], F32, tag="ot", name="ot")
                nc.vector.tensor_scalar(ot[:], p2[:], 1.0 / SW,
                                        bf2_sb[:, mt:mt + 1],
                                        mybir.AluOpType.mult,
                                        mybir.AluOpType.add)
                nc.sync.dma_start(out_fm[:, mt, jsl], ot[:])

            slot = 0
            for mt in range(6):
                w2 = wpool.tile([128, 2, GB, 2, 128], FP8, tag="w2", name="w2")
                nc.sync.dma_start(w2[:], w_f2a[mt])
                for j in range(nch - 1):
                    fc2_group(mt, j, w2)
                    next(fc1_last, None)
                    if slot % 3 == 0:
                        next(fc1_last, None)
                    slot += 1
            for _ in fc1_last:
                pass
            for mt in range(6):
                w2 = wpool.tile([128, 2, GB, 2, 128], FP8, tag="w2", name="w2")
                nc.sync.dma_start(w2[:], w_f2a[mt])
                fc2_group(mt, nch - 1, w2)

    nc.compile()
    return nc


_NC_CACHE = {}


def _get_nc(bl=BL):
    if bl not in _NC_CACHE:
        _NC_CACHE[bl] = build_nc(bl)
    return _NC_CACHE[bl]


def _q8(a):
    import ml_dtypes
    return np.asarray(a, np.float32).astype(ml_dtypes.float8_e4m3)


def _qb(a):
    import ml_dtypes
    return np.asarray(a, np.float32).astype(ml_dtypes.bfloat16)


def _stationary(w, nmt):
    o, kc = w.shape
    nb = kc // 256
    return np.ascontiguousarray(
        w.reshape(nmt, 128, nb, 2, 128).transpose(0, 4, 2, 3, 1))


def _host_prep(x, prompt, ln1_w, ln1_b, qkv_w, qkv_b, proj_w, proj_b,
               ln2_w, ln2_b, fc1_w, fc1_b, fc2_w, fc2_b, bl=BL, ncores=NCORES):
    import ml_dtypes
    f8 = np.float64
    w_qk = (f8(qkv_w[:2 * C]) * f8(ln1_w)).astype(np.float32)
    b_qkf = (f8(qkv_b[:2 * C]) + f8(qkv_w[:2 * C]) @ f8(ln1_b)).astype(np.float32)
    w_v = (f8(qkv_w[2 * C:]) * f8(ln1_w)).astype(np.float32)
    b_v = (f8(qkv_b[2 * C:]) + f8(qkv_w[2 * C:]) @ f8(ln1_b)).astype(np.float32)
    b_pr = (f8(proj_b) + f8(proj_w) @ f8(b_v)).astype(np.float32)
    w_f1 = (f8(fc1_w) * f8(ln2_w)).astype(np.float32)
    b_f1f = (f8(fc1_b) + f8(fc1_w) @ f8(ln2_b)).astype(np.float32)

    perm = np.zeros(C, dtype=np.int64)
    for mt6 in range(6):
        hp, jd = mt6 // 2, mt6 % 2
        for pr in range(128):
            hq, dl = pr // 32, pr % 32
            perm[mt6 * 128 + pr] = (hp * 4 + hq) * 64 + jd * 32 + dl

    wq = _stationary(_q8(w_qk[:C][perm] * SW), 6)
    wk = _stationary(_q8(w_qk[C:][perm] * SW), 6)
    w_qk8 = np.ascontiguousarray(
        np.concatenate([wq, wk], axis=0).transpose(1, 0, 2, 3, 4))
    b_qk_p = np.concatenate([b_qkf[:C][perm].reshape(6, 128),
                             b_qkf[C:][perm].reshape(6, 128)], axis=0).T.copy()

    wv8 = _q8(w_v * SW)
    w_v8 = np.ascontiguousarray(wv8.T.reshape(CB, 2, 128, C).transpose(2, 0, 1, 3))

    w_pr8 = np.ascontiguousarray(
        _stationary(_q8(np.float32(proj_w) * SW), 6).transpose(1, 0, 2, 3, 4))

    wf1s = np.float32(w_f1) * SW
    wf18 = _q8(wf1s)
    wf1r = _q8(wf1s - np.float32(wf18))
    w_f1a = np.stack([_stationary(wf18, 24), _stationary(wf1r, 24)], axis=1)
    w_f1a = np.ascontiguousarray(w_f1a.transpose(2, 0, 1, 3, 4, 5))

    wf2s = np.float32(fc2_w) * SW
    wf28 = _q8(wf2s)
    wf2r = _q8(wf2s - np.float32(wf28))
    w_f2a = np.stack([_stationary(wf28, 6), _stationary(wf2r, 6)], axis=1)
    w_f2a = np.ascontiguousarray(w_f2a.transpose(0, 2, 1, 3, 4, 5))

    b_f1a = np.float32(b_f1f).reshape(24, 128).T.copy()
    b_f2a = np.float32(fc2_b).reshape(6, 128).T.copy()
    ident = np.eye(128, dtype=ml_dtypes.bfloat16)

    shared = dict(w_qk8=w_qk8, w_v8=w_v8, w_pr8=w_pr8, w_f1a=w_f1a, w_f2a=w_f2a,
                  b_qk=b_qk_p, b_f1=b_f1a, b_f2=b_f2a, ident_d=ident)

    T = bl * N
    x = np.float32(x).reshape(ncores, bl, N, C)
    prompt = np.float32(prompt).reshape(ncores, bl, P, 2, H, Dh)
    in_maps = []
    for c in range(ncores):
        xc = x[c].reshape(T, C)
        x_fmc = np.ascontiguousarray(
            _qb(xc.T).reshape(6, 128, T).transpose(1, 0, 2))
        xresc = np.ascontiguousarray(
            _qb((xc + b_pr).T * SW).reshape(6, 128, T).transpose(1, 0, 2))
        pk = np.float32(_q8(prompt[c, :, :, 0]))
        pk = pk.reshape(bl, P, CB, 4, 2, 32).transpose(3, 5, 2, 4, 0, 1)
        kp8 = np.zeros((128, CB, 2, bl, NPAD), dtype=ml_dtypes.float8_e4m3)
        kp8[:, :, :, :, :P] = _q8(pk.reshape(128, CB, 2, bl, P))
        vp8 = np.zeros((NPAD, bl, H, 68), dtype=ml_dtypes.float8_e4m3)
        vp8[:P, :, :, :Dh] = _q8(prompt[c, :, :, 1].transpose(1, 0, 2, 3))
        vp8[:P, :, :, Dh] = 1.0
        in_maps.append(dict(x_fm=x_fmc, xres=xresc, kp8=kp8, vp8=vp8, **shared))
    return in_maps


def run_sharded(inputs, bl=BL, ncores=NCORES, **spmd_kwargs):
    in_maps = _host_prep(**inputs, bl=bl, ncores=ncores)
    nc = _get_nc(bl)
    res = bass_utils.run_bass_kernel_spmd(nc, in_maps, core_ids=list(range(ncores)),
                                          **spmd_kwargs)
    T = bl * N
    outs = [r["out_fm"].transpose(1, 0, 2).reshape(C, T).T.reshape(bl, N, C)
            for r in res.results]
    return np.concatenate(outs, axis=0).astype(np.float32), res


def kernel(**inputs):
    out, _ = run_sharded(inputs, bl=BL, ncores=NCORES)
    return out
